# revision 11
# baseline (speedup 1.0000x reference)
"""Trainium2 Bass kernel for nn_Expand_36610301231376 — v2 redesign.

kernel(**inputs) takes the FULL unsharded inputs (as in reference.setup_inputs)
and returns the FULL (16, 512, 56, 56) float32 output.

Strategy: pure data parallel over batch B=16 across 8 NeuronCores (2 batches
per core). Within a core, 3136 tokens are processed in 7 chunks of 448 (8 image
rows), attention on 2-row blocks of 112 tokens. Key design points vs v1:

- LN1 statistics via the Gram trick: sum_d xe^2 = w1^T (x x^T) w1, computed
  token-major on the PE (H = G @ w1 per chunk) so the nonlinear finalize runs
  on fast [112,x] columns instead of [1,448] rows. Mean via an extra xsum
  column appended to G.
- LN2/value path is token-major: y is DMA'd in BOTH layouts (bf16), stats are
  free-dim reductions, (y-m)*r is a single per-partition-scalar op, and the
  attention value matmul uses the token-major core directly — no PE transposes
  of ny and no ones-matmul stats.
- k is computed from RAW y (LN folded): k = r2b*(Wk_g2 @ y + nuk x mu2row) + ck.
- cq (the constant part of q) is folded into the score matmuls as an extra
  stationary accumulation, so q = r1b * (Wq_g1 @ xe + nuq x mu1row) in one op.
- All DMA in bf16 (y both layouts, out, consts); all matmuls bf16 with fp32
  accumulation.
- Per-chunk phases of the two batches are interleaved to keep the PE dense and
  warm (HAM clock gate).
"""
import sys

if "/opt/trn_rl_repo" not in sys.path:
    sys.path.insert(0, "/opt/trn_rl_repo")

import numpy as np
import orjson

# ----------------------------------------------------------------------------
# BIR post-pass: this container's walrus build supports only ONE sync-wait per
# instruction; split multi-wait instructions into single-wait NoOps.
# ----------------------------------------------------------------------------
_wcounter = [0]


def _split_block(instructions):
    out, changed = [], False
    for inst in instructions:
        si = inst.get("sync_info")
        waits = (si or {}).get("on_wait") or []
        if len(waits) > 1:
            changed = True
            for w in waits[:-1]:
                _wcounter[0] += 1
                nop = {
                    "engine": inst["engine"], "ins": [], "outs": [],
                    "name": f"I-wsplit-{_wcounter[0]}", "opcode": "NoOp",
                    "sync_info": {"on_update": [], "on_wait": [w]},
                }
                if "debug" in inst:
                    nop["debug"] = inst["debug"]
                out.append(nop)
            si["on_wait"] = [waits[-1]]
        out.append(inst)
    return out, changed


def _split_multi_waits_json(bir_json: bytes) -> bytes:
    m = orjson.loads(bir_json)
    changed = False
    for fn in m.get("functions", []):
        for blk in fn.get("blocks", []):
            insts = blk.get("instructions")
            if insts:
                blk["instructions"], ch = _split_block(insts)
                changed = changed or ch
    return orjson.dumps(m) if changed else bir_json


def _install_patch():
    import concourse.bass as bass

    if getattr(bass.Bass, "_wait_split_installed", False):
        return
    orig = bass.Bass.to_json_bytes

    def to_json_bytes(self):
        return _split_multi_waits_json(orig(self))

    bass.Bass.to_json_bytes = to_json_bytes
    bass.Bass._wait_split_installed = True


# ----------------------------------------------------------------------------
# Problem constants (hardcoded from the problem spec)
# ----------------------------------------------------------------------------
B = 16
N_CORES = 8
B_LOC = B // N_CORES
T_LEN, T_DIM = 149, 768
H = W = 56
S_DIM = 512
N_TOK = H * W           # 3136
CH = 448                # tokens per chunk (8 image rows)
NCHUNK = N_TOK // CH    # 7
NBLK = CH // 112        # 4 two-row attention blocks per chunk
NG = N_TOK // 112       # 28 token groups of 112
EPS = 1e-5


# ----------------------------------------------------------------------------
# Device program
# ----------------------------------------------------------------------------
def _build_program():
    import concourse.bass as bass
    import concourse.tile as tile
    from concourse import mybir

    F32 = mybir.dt.float32
    BF16 = mybir.dt.bfloat16
    AF = mybir.ActivationFunctionType
    OP = mybir.AluOpType
    AX = mybir.AxisListType

    nc = bass.Bass(trn_type="TRN2", target_bir_lowering=False, debug=False)
    din = {}
    for name, shape, dt_ in [
        ("x0", (128, B_LOC, T_DIM), BF16), ("x1", (32, B_LOC, T_DIM), BF16),
        ("xT", (128, 6, B_LOC, T_LEN), BF16),
        ("w1t", (128, 2, N_TOK), BF16),
        ("w1tm", (112, NG, T_LEN), BF16),
        ("wqgt", (128, 6, S_DIM), BF16), ("nuq", (1, S_DIM), BF16),
        ("wkgt", (128, 4, S_DIM), BF16), ("nuk32", (33, S_DIM), BF16),
        ("cq", (128, 4, N_TOK), BF16),
        ("ck", (128, 4, N_TOK), BF16),
        ("pe2tm", (112, NG, S_DIM), BF16),
        ("mst2", (2, 112), BF16), ("mmv2", (2, 112), BF16),
        ("ones", (128, 128), BF16), ("ident", (128, 128), BF16),
        ("ytm", (B_LOC, 112, NG, S_DIM), BF16),
        ("ycm", (B_LOC, 128, 4, N_TOK), BF16),
    ]:
        din[name] = nc.dram_tensor(name, list(shape), dt_, kind="ExternalInput").ap()
    dout = nc.dram_tensor("out", [B_LOC, 128, 4, N_TOK], BF16,
                          kind="ExternalOutput").ap()

    from contextlib import ExitStack

    with nc.allow_low_precision(reason="bf16 matmul operands, fp32 accumulate"), \
         tile.TileContext(nc) as tc, ExitStack() as ctx:
        singles = ctx.enter_context(tc.tile_pool(name="singles", bufs=1))
        io2 = ctx.enter_context(tc.tile_pool(name="io2", bufs=2))
        io3 = ctx.enter_context(tc.tile_pool(name="io3", bufs=3))
        wk2 = ctx.enter_context(tc.tile_pool(name="wk2", bufs=2))
        sc2 = ctx.enter_context(tc.tile_pool(name="sc2", bufs=2))
        # PSUM pools — total banks must be <= 8
        ps_mm = ctx.enter_context(tc.tile_pool(name="ps_mm", bufs=3, space="PSUM"))
        ps_at = ctx.enter_context(tc.tile_pool(name="ps_at", bufs=2, space="PSUM"))
        ps_av = ctx.enter_context(tc.tile_pool(name="ps_av", bufs=2, space="PSUM"))
        ps_xp = ctx.enter_context(tc.tile_pool(name="ps_xp", bufs=1, space="PSUM"))

        def load(name, shape, dt_):
            t = singles.tile(list(shape), dt_, tag=name)
            nc.sync.dma_start(out=t, in_=din[name])
            return t

        x0 = load("x0", (128, B_LOC, T_DIM), BF16)
        x1 = load("x1", (32, B_LOC, T_DIM), BF16)
        xT = load("xT", (128, 6, B_LOC, T_LEN), BF16)
        w1t = load("w1t", (128, 2, N_TOK), BF16)
        w1tm = load("w1tm", (112, NG, T_LEN), BF16)
        wqgt = load("wqgt", (128, 6, S_DIM), BF16)
        nuq = load("nuq", (1, S_DIM), BF16)
        wkgt = load("wkgt", (128, 4, S_DIM), BF16)
        nuk32 = load("nuk32", (33, S_DIM), BF16)
        mst2 = load("mst2", (2, 112), BF16)
        mmv2 = load("mmv2", (2, 112), BF16)
        ones = load("ones", (128, 128), BF16)
        ident = load("ident", (128, 128), BF16)
        ones_row = ones[0:1, :]
        eps112 = singles.tile([112, 1], mybir.dt.float32, name="eps112")
        nc.vector.memset(eps112, EPS)

        x_k = [(x0, 128), (x1, 21)]

        # ------------------------------------------------------------------
        # Per-batch G gram (+xsum column): g_s[b] = [x x^T | xsum] bf16
        # g0: [128,150] rows l=0..127 ; g1: [32,150] rows l=128..148
        # ------------------------------------------------------------------
        g_s = []
        for b in range(B_LOC):
            g0 = singles.tile([128, T_LEN + 1], BF16, tag=f"g0_{b}")
            g1 = singles.tile([32, T_LEN + 1], BF16, tag=f"g1_{b}")
            pG0 = ps_xp.tile([128, T_LEN], F32, tag="xp")
            for d in range(6):
                nc.tensor.matmul(pG0, xT[:, d, b, 0:128], xT[:, d, b, :],
                                 start=(d == 0), stop=(d == 5))
            nc.vector.tensor_copy(out=g0[:, 0:T_LEN], in_=pG0)
            pG1 = ps_xp.tile([32, T_LEN], F32, tag="xp")
            for d in range(6):
                nc.tensor.matmul(pG1[0:21, :], xT[:, d, b, 128:T_LEN],
                                 xT[:, d, b, :], start=(d == 0), stop=(d == 5))
            nc.scalar.activation(out=g1[0:21, 0:T_LEN], in_=pG1[0:21, :],
                                 func=AF.Copy)
            nc.vector.tensor_reduce(out=g0[:, T_LEN:T_LEN + 1],
                                    in_=x0[:, b, :], axis=AX.X, op=OP.add)
            nc.vector.tensor_reduce(out=g1[0:21, T_LEN:T_LEN + 1],
                                    in_=x1[0:21, b, :], axis=AX.X, op=OP.add)
            g_s.append((g0, g1))

        # ------------------------------------------------------------------
        # Phases (st = per-(chunk, batch) state dict)
        # ------------------------------------------------------------------
        def phase_dma(st):
            b, ich, cols = st["b"], st["ich"], st["cols"]
            ytm_t = io3.tile([112, NBLK, S_DIM], BF16, tag="ytm")
            nc.sync.dma_start(out=ytm_t, in_=din["ytm"][b, :, 4 * ich:4 * ich + 4, :])
            ycm_t = io3.tile([128, 4, CH], BF16, tag="ycm")
            nc.sync.dma_start(out=ycm_t, in_=din["ycm"][b, :, :, cols])
            st["ytm"], st["ycm"] = ytm_t, ycm_t

        def chunk_dma(ich, cols):
            cq_t = io2.tile([128, 4, CH], BF16, tag="cq")
            nc.sync.dma_start(out=cq_t, in_=din["cq"][:, :, cols])
            ck_t = io2.tile([128, 4, CH], BF16, tag="ck")
            nc.sync.dma_start(out=ck_t, in_=din["ck"][:, :, cols])
            pe2_t = io2.tile([112, NBLK, S_DIM], BF16, tag="pe2")
            nc.sync.dma_start(out=pe2_t, in_=din["pe2tm"][:, 4 * ich:4 * ich + 4, :])
            return cq_t, ck_t, pe2_t

        def phase_xe(st):
            b, cols = st["b"], st["cols"]
            xe = wk2.tile([128, 6, CH], BF16, tag="xe")
            for m in range(6):
                pxe = ps_mm.tile([128, CH], F32, tag="mm")
                for ik, (xt, kv) in enumerate(x_k):
                    nc.tensor.matmul(
                        pxe, xt[:kv, b, m * 128:(m + 1) * 128],
                        w1t[:kv, ik, cols], start=(ik == 0), stop=(ik == 1))
                if m % 2 == 0:
                    nc.vector.tensor_copy(out=xe[:, m, :], in_=pxe)
                else:
                    nc.scalar.activation(out=xe[:, m, :], in_=pxe, func=AF.Copy)
            st["xe"] = xe

        def phase_stats1(st):
            # H = w1_tok^T @ [G|xsum] token-major; qs1/mu1 per token column
            b, ich = st["b"], st["ich"]
            g0, g1 = g_s[b]
            S = st["S"]
            qs1 = st["qs1"]
            for blk in range(NBLK):
                g = 4 * ich + blk
                tok = slice(ich * CH + blk * 112, ich * CH + (blk + 1) * 112)
                Hp = ps_xp.tile([112, T_LEN + 1], F32, tag="xp")
                nc.tensor.matmul(Hp, w1t[:, 0, tok], g0,
                                 start=True, stop=False)
                nc.tensor.matmul(Hp, w1t[:21, 1, tok], g1[0:21, :],
                                 start=False, stop=True)
                scr = sc2.tile([112, T_LEN], F32, tag="scr")
                nc.vector.tensor_tensor(out=scr, in0=Hp[:, 0:T_LEN],
                                        in1=w1tm[:, g, :], op=OP.mult)
                nc.vector.tensor_reduce(out=qs1[:, blk:blk + 1], in_=scr,
                                        axis=AX.X, op=OP.add)
                # mu1 = Hp[:,149] / 768  -> S[:, blk, 0]
                nc.vector.tensor_scalar(
                    out=S[:, blk, 0:1], in0=Hp[:, T_LEN:T_LEN + 1],
                    scalar1=float(1.0 / T_DIM), scalar2=None, op0=OP.mult)

        def phase_stats2(st):
            # LN2 stats from token-major y: sums on DVE, sumsq on scalar engine
            ytm_t = st["ytm"]
            S = st["S"]
            qs2, sum2 = st["qs2"], st["sum2"]
            for blk in range(NBLK):
                nc.vector.tensor_reduce(out=sum2[:, blk:blk + 1],
                                        in_=ytm_t[:, blk, :], axis=AX.X, op=OP.add)
                scr2 = sc2.tile([112, S_DIM], BF16, tag="scr2")
                nc.scalar.activation(out=scr2, in_=ytm_t[:, blk, :],
                                     func=AF.Square,
                                     accum_out=qs2[:, blk:blk + 1])
            # mu2 = sum2/512 -> f32 col + bf16 row-source in S[:,:,32]
            mu2c = st["mu2c"]
            nc.vector.tensor_scalar(out=mu2c, in0=sum2,
                                    scalar1=float(1.0 / S_DIM), scalar2=None,
                                    op0=OP.mult)
            nc.scalar.activation(out=S[:, :, 32], in_=mu2c, func=AF.Copy)

        def phase_finalize(st):
            # var = qs/D - mu^2 ; r = 1/sqrt(var+eps); columns [112, 4]
            S = st["S"]
            qs1, qs2 = st["qs1"], st["qs2"]
            mu2c, r1c, r2c = st["mu2c"], st["r1c"], st["r2c"]
            msq = sc2.tile([112, 4], F32, tag="msq")
            var = sc2.tile([112, 4], F32, tag="var")
            sd = sc2.tile([112, 4], F32, tag="sd")
            # LN1 (r1 stays a column; mu1 lives in S[:,:,0] for the transpose)
            nc.vector.tensor_tensor(out=msq, in0=S[:, :, 0], in1=S[:, :, 0],
                                    op=OP.mult)
            nc.vector.scalar_tensor_tensor(
                out=var, in0=qs1, scalar=float(1.0 / T_DIM), in1=msq,
                op0=OP.mult, op1=OP.subtract)
            nc.scalar.activation(out=sd, in_=var, func=AF.Sqrt, bias=eps112)
            nc.vector.reciprocal(out=r1c, in_=sd)
            # LN2
            msq2 = sc2.tile([112, 4], F32, tag="msq2")
            var2 = sc2.tile([112, 4], F32, tag="var2")
            sd2 = sc2.tile([112, 4], F32, tag="sd2")
            nc.vector.tensor_tensor(out=msq2, in0=mu2c, in1=mu2c, op=OP.mult)
            nc.vector.scalar_tensor_tensor(
                out=var2, in0=qs2, scalar=float(1.0 / S_DIM), in1=msq2,
                op0=OP.mult, op1=OP.subtract)
            nc.scalar.activation(out=sd2, in_=var2, func=AF.Sqrt, bias=eps112)
            nc.vector.reciprocal(out=r2c, in_=sd2)
            nc.scalar.activation(out=S[:, :, 64], in_=r2c, func=AF.Copy)
            # negmr2 = -mu2 * r2 (f32 cols, used as activation bias for core)
            negmr2 = st["negmr2"]
            nc.vector.scalar_tensor_tensor(
                out=negmr2, in0=mu2c, scalar=-1.0, in1=r2c,
                op0=OP.mult, op1=OP.mult)

        def phase_rows(st):
            # transpose stat columns to rows at base partitions 0/32/64:
            # row 0: mu1, row 32: mu2, row 64: r2
            S = st["S"]
            rowsP = ps_xp.tile([65, CH], BF16, tag="xp")
            for blk in range(NBLK):
                tb = slice(blk * 112, (blk + 1) * 112)
                nc.tensor.transpose(rowsP[:, tb], S[:, blk, :],
                                    ident[0:112, 0:112])
            rows = sc2.tile([65, CH], BF16, tag="rows")
            nc.scalar.activation(out=rows, in_=rowsP, func=AF.Copy)
            st["rows"] = rows
            prb2 = ps_mm.tile([128, CH], F32, tag="mm")
            nc.tensor.matmul(prb2, ones[64:65, :], rows[64:65, :],
                             start=True, stop=True)
            r2b = wk2.tile([128, CH], BF16, tag="r2b")
            nc.vector.tensor_copy(out=r2b, in_=prb2)
            st["r2b"] = r2b

        def phase_core(st):
            # core = (ytm * r2 + negmr2); v = core (+ pe2) token-major
            ytm_t, negmr2 = st["ytm"], st["negmr2"]
            r2c = st["r2c"]
            pe2_t = st["pe2_t"]
            core = wk2.tile([112, NBLK, S_DIM], BF16, tag="core")
            v = wk2.tile([112, NBLK, S_DIM], BF16, tag="v")
            for blk in range(NBLK):
                nc.scalar.activation(out=core[:, blk, :], in_=ytm_t[:, blk, :],
                                     func=AF.Identity,
                                     scale=r2c[:, blk:blk + 1],
                                     bias=negmr2[:, blk:blk + 1])
                nc.gpsimd.tensor_add(out=v[:, blk, :], in0=core[:, blk, :],
                                     in1=pe2_t[:, blk, :])
            st["v"] = v

        def phase_q(st):
            xe, rows = st["xe"], st["rows"]
            q = wk2.tile([128, 4, CH], BF16, tag="q")
            for oc in range(4):
                pq = ps_mm.tile([128, CH], F32, tag="mm")
                for kc in range(6):
                    nc.tensor.matmul(
                        pq, wqgt[:, kc, oc * 128:(oc + 1) * 128],
                        xe[:, kc, :], start=(kc == 0), stop=False)
                nc.tensor.matmul(pq, nuq[:, oc * 128:(oc + 1) * 128],
                                 rows[0:1, :], start=False, stop=True)
                if oc % 2 == 0:
                    nc.vector.tensor_copy(out=q[:, oc, :], in_=pq)
                else:
                    nc.scalar.activation(out=q[:, oc, :], in_=pq, func=AF.Copy)
            st["q"] = q

        def phase_k(st):
            ycm_t, rows, r2b, ck_t = st["ycm"], st["rows"], st["r2b"], st["ck_t"]
            k = wk2.tile([128, 4, CH], BF16, tag="k")
            for oc in range(4):
                pk = ps_mm.tile([128, CH], F32, tag="mm")
                for kc in range(4):
                    nc.tensor.matmul(
                        pk, wkgt[:, kc, oc * 128:(oc + 1) * 128],
                        ycm_t[:, kc, :], start=(kc == 0), stop=False)
                nc.tensor.matmul(pk, nuk32[32:33, oc * 128:(oc + 1) * 128],
                                 rows[32:33, :], start=False, stop=True)
                k1 = sc2.tile([128, CH], BF16, tag="k1")
                nc.vector.tensor_tensor(out=k1, in0=pk, in1=r2b, op=OP.mult)
                nc.vector.tensor_tensor(out=k[:, oc, :], in0=k1,
                                        in1=ck_t[:, oc, :], op=OP.add)
            st["k"] = k

        def phase_scores(st):
            q, k, cq_t = st["q"], st["k"], st["cq_t"]
            r1c = st["r1c"]
            den = st["den"]
            e4 = wk2.tile([112, NBLK, 112], BF16, tag="e4")
            pscs = []
            for blk in range(NBLK):
                tb = slice(blk * 112, (blk + 1) * 112)
                psc = ps_at.tile([112, 2, 112], F32, tag="at")
                for oc in range(4):
                    nc.tensor.matmul(psc[:, 0, :], q[:, oc, tb], k[:, oc, tb],
                                     start=(oc == 0), stop=(oc == 3))
                for oc in range(4):
                    nc.tensor.matmul(psc[:, 1, :], cq_t[:, oc, tb],
                                     k[:, oc, tb], start=(oc == 0), stop=False)
                nc.tensor.matmul(psc[:, 1, :], mst2, mmv2,
                                 start=False, stop=True)
                sa_s = sc2.tile([112, 112], BF16, tag="sa_s", name="sa_s")
                nc.vector.tensor_scalar(out=sa_s, in0=psc[:, 0, :],
                                        scalar1=r1c[:, blk:blk + 1],
                                        scalar2=None, op0=OP.mult)
                nc.vector.tensor_tensor(out=psc[:, 1, :], in0=psc[:, 1, :],
                                        in1=sa_s, op=OP.add)
                pscs.append(psc)
            for blk in range(NBLK):
                nc.scalar.activation(out=e4[:, blk, :], in_=pscs[blk][:, 1, :],
                                     func=AF.Exp, accum_out=den[:, blk:blk + 1])
            st["e4"] = e4

        def phase_attn(st):
            e4, den = st["e4"], st["den"]
            dinv = sc2.tile([112, 4], F32, tag="dinv")
            nc.vector.reciprocal(out=dinv, in_=den)
            attnT = wk2.tile([112, NBLK, 112], BF16, tag="attnT")
            for blk in range(NBLK):
                attn = sc2.tile([112, 112], BF16, tag="attn")
                nc.vector.tensor_scalar(out=attn, in0=e4[:, blk, :],
                                        scalar1=dinv[:, blk:blk + 1],
                                        scalar2=None, op0=OP.mult)
                pat = ps_at.tile([112, 112], BF16, tag="at")
                nc.tensor.transpose(pat, attn, ident[0:112, 0:112])
                if blk % 2 == 0:
                    nc.vector.tensor_copy(out=attnT[:, blk, :], in_=pat)
                else:
                    nc.scalar.activation(out=attnT[:, blk, :], in_=pat,
                                         func=AF.Copy)
            st["attnT"] = attnT

        def phase_av(st):
            b, cols = st["b"], st["cols"]
            v, attnT, ycm_t = st["v"], st["attnT"], st["ycm"]
            out_t = io2.tile([128, 4, CH], BF16, tag="out")
            for blk in range(NBLK):
                tb = slice(blk * 112, (blk + 1) * 112)
                pav = ps_av.tile([128, 4, 112], F32, tag="av")
                for co in range(4):
                    nc.tensor.matmul(pav[:, co, :],
                                     v[:, blk, co * 128:(co + 1) * 128],
                                     attnT[:, blk, :], start=True, stop=True)
                nc.vector.tensor_tensor(out=out_t[:, :, tb], in0=pav,
                                        in1=ycm_t[:, :, tb], op=OP.add)
            nc.sync.dma_start(out=dout[b, :, :, cols], in_=out_t)

        def new_state(b, ich, cols, cq_t, ck_t, pe2_t):
            st = {"b": b, "ich": ich, "cols": cols,
                  "cq_t": cq_t, "ck_t": ck_t, "pe2_t": pe2_t}
            st["S"] = sc2.tile([112, NBLK, 65], BF16, tag="S", name="S")
            st["mu2c"] = sc2.tile([112, 4], F32, tag="mu2c", name="mu2c")
            st["r1c"] = sc2.tile([112, 4], F32, tag="r1c", name="r1c")
            st["r2c"] = sc2.tile([112, 4], F32, tag="r2c", name="r2c")
            st["qs1"] = sc2.tile([112, 4], F32, tag="qs1", name="qs1")
            st["qs2"] = sc2.tile([112, 4], F32, tag="qs2", name="qs2")
            st["sum2"] = sc2.tile([112, 4], F32, tag="sum2", name="sum2")
            st["negmr2"] = sc2.tile([112, 4], F32, tag="negmr2", name="negmr2")
            st["den"] = sc2.tile([112, 4], F32, tag="den", name="den")
            return st

        # ------------------------------------------------------------------
        # Main loop: software pipeline over 14 (chunk, batch) units.
        # Unit i's attention tail (scores/softmax/attnT/av) is interleaved
        # into unit i+1's projection phases so the PE never starves.
        # ------------------------------------------------------------------
        units = [(ich, b) for ich in range(NCHUNK) for b in range(B_LOC)]
        chunk_tiles = {}
        states = [None] * len(units)

        def ensure_state(i):
            if i >= len(units) or states[i] is not None:
                return
            ich, b = units[i]
            cols = slice(ich * CH, (ich + 1) * CH)
            if ich not in chunk_tiles:
                chunk_tiles[ich] = chunk_dma(ich, cols)
            cq_t, ck_t, pe2_t = chunk_tiles[ich]
            st = new_state(b, ich, cols, cq_t, ck_t, pe2_t)
            phase_dma(st)
            states[i] = st

        def tail_a(st):
            phase_scores(st)

        def tail_b(st):
            phase_attn(st)

        def tail_c(st):
            phase_av(st)

        prev = None
        for i in range(len(units)):
            ensure_state(i)
            ensure_state(i + 1)
            ensure_state(i + 2)
            st = states[i]
            phase_xe(st)
            if prev is not None:
                tail_a(prev)
            phase_stats1(st)
            phase_stats2(st)
            phase_finalize(st)
            phase_rows(st)
            phase_core(st)
            phase_q(st)
            if prev is not None:
                tail_b(prev)
            phase_k(st)
            if prev is not None:
                tail_c(prev)
            prev = st
        tail_a(prev)
        tail_b(prev)
        tail_c(prev)
    return nc


# ----------------------------------------------------------------------------
# Host-side preparation
# ----------------------------------------------------------------------------
def _make_const_inputs(W_conv1, b_conv1, ln1_g, ln1_b, ln2_g, ln2_b,
                       pe_wave, pe_spec, Wq, bq, Wk, bk):
    import ml_dtypes
    f = np.float32
    bf = ml_dtypes.bfloat16
    s = np.float32(S_DIM) ** np.float32(-0.25)

    w1T = W_conv1.T.astype(f)                       # (149, N_TOK)
    w1t = np.zeros((128, 2, N_TOK), dtype=f)
    w1t[:, 0, :] = w1T[:128]
    w1t[:21, 1, :] = w1T[128:]

    # token-major w1 for the Gram reduce: [112, 28, 149]
    w1tm = W_conv1.astype(f).reshape(NG, 112, T_LEN).transpose(1, 0, 2).copy()

    wqg = (Wq * ln1_g[None, :]).astype(f) * s
    wqgt = wqg.T.reshape(6, 128, S_DIM).transpose(1, 0, 2).copy()
    nuq = (-(Wq @ ln1_g) * s).astype(f)[None, :]

    pe_w = pe_wave.reshape(T_DIM, N_TOK).astype(f)
    cq = (Wq @ (ln1_b[:, None] + pe_w)).astype(f) * s + (bq[:, None] * s).astype(f)
    cq = cq.reshape(4, 128, N_TOK).transpose(1, 0, 2).copy()

    wkg = (Wk * ln2_g[None, :]).astype(f) * s
    wkgt = wkg.T.reshape(4, 128, S_DIM).transpose(1, 0, 2).copy()
    nuk = (-(Wk @ ln2_g) * s).astype(f)[None, :]

    pe_s2 = pe_spec.reshape(S_DIM, N_TOK).astype(f)
    ck = (Wk @ (ln2_b[:, None] + pe_s2)).astype(f) * s + (bk[:, None] * s).astype(f)
    ck = ck.reshape(4, 128, N_TOK).transpose(1, 0, 2).copy()

    # token-major (pe_spec + ln2_b): [112, 28, 512]
    pe2tm = (pe_s2 + ln2_b[:, None]).T.astype(f)     # (N_TOK, 512)
    pe2tm = pe2tm.reshape(NG, 112, S_DIM).transpose(1, 0, 2).copy()

    # mask = -1e30*(u0 x u1 + u1 x u0) as two rank-1 bf16 factors
    u0 = np.zeros((112,), dtype=f); u0[:56] = 1.0
    u1 = np.zeros((112,), dtype=f); u1[56:] = 1.0
    mst2 = np.zeros((2, 112), dtype=f)
    mmv2 = np.zeros((2, 112), dtype=f)
    mst2[0] = -1e30 * u0; mmv2[0] = u1
    mst2[1] = -1e30 * u1; mmv2[1] = u0

    nuk32 = np.zeros((33, S_DIM), dtype=f)
    nuk32[32] = nuk[0]

    return {
        "w1t": w1t.astype(bf), "w1tm": w1tm.astype(bf),
        "wqgt": wqgt.astype(bf), "nuq": nuq.astype(bf),
        "wkgt": wkgt.astype(bf), "nuk32": nuk32.astype(bf),
        "cq": cq.astype(bf), "ck": ck.astype(bf),
        "pe2tm": pe2tm.astype(bf),
        "mst2": mst2.astype(bf), "mmv2": mmv2.astype(bf),
        "ones": np.ones((128, 128), dtype=bf),
        "ident": np.eye(128, dtype=bf),
    }


def _make_core_inputs(consts, x_shard, y_shard):
    import ml_dtypes
    bf = ml_dtypes.bfloat16
    f = np.float32
    x_shard = np.asarray(x_shard, dtype=f)
    y_shard = np.asarray(y_shard, dtype=f)
    x0 = x_shard[:, :128, :].transpose(1, 0, 2).astype(bf).copy()
    x1 = np.zeros((32, B_LOC, T_DIM), dtype=bf)
    x1[:21] = x_shard[:, 128:, :].transpose(1, 0, 2).astype(bf)
    # xT: [128, 6, B_LOC, 149]
    xT = x_shard.transpose(2, 0, 1).reshape(6, 128, B_LOC, T_LEN)
    xT = xT.transpose(1, 0, 2, 3).astype(bf).copy()
    yf = y_shard.reshape(B_LOC, S_DIM, N_TOK)
    ycm = yf.reshape(B_LOC, 4, 128, N_TOK).transpose(0, 2, 1, 3).astype(bf).copy()
    ytm = yf.transpose(0, 2, 1).reshape(B_LOC, NG, 112, S_DIM)
    ytm = ytm.transpose(0, 2, 1, 3).astype(bf).copy()
    m = {"x0": x0, "x1": x1, "xT": xT, "ycm": ycm, "ytm": ytm}
    m.update(consts)
    return m


_cached_nc = [None]


def kernel(x, y, W_conv1, b_conv1, ln1_g, ln1_b, ln2_g, ln2_b,
           pe_wave, pe_spec, Wq, bq, Wk, bk):
    _install_patch()
    from concourse.bass_utils import run_bass_kernel_spmd

    x = np.asarray(x, dtype=np.float32)
    y = np.asarray(y, dtype=np.float32)
    consts = _make_const_inputs(
        np.asarray(W_conv1, np.float32), np.asarray(b_conv1, np.float32),
        np.asarray(ln1_g, np.float32), np.asarray(ln1_b, np.float32),
        np.asarray(ln2_g, np.float32), np.asarray(ln2_b, np.float32),
        np.asarray(pe_wave, np.float32), np.asarray(pe_spec, np.float32),
        np.asarray(Wq, np.float32), np.asarray(bq, np.float32),
        np.asarray(Wk, np.float32), np.asarray(bk, np.float32))
    in_maps = [
        _make_core_inputs(consts, x[B_LOC * i:B_LOC * (i + 1)],
                          y[B_LOC * i:B_LOC * (i + 1)])
        for i in range(N_CORES)
    ]

    if _cached_nc[0] is None:
        _cached_nc[0] = _build_program()
    nc = _cached_nc[0]

    res = run_bass_kernel_spmd(nc, in_maps, core_ids=list(range(N_CORES)))
    outs = []
    for i in range(N_CORES):
        o = np.asarray(res.results[i]["out"]).astype(np.float32)
        outs.append(o.transpose(0, 2, 1, 3).reshape(B_LOC, S_DIM, H, W))
    return np.concatenate(outs, axis=0).astype(np.float32)
